# revision 62
# baseline (speedup 1.0000x reference)
"""Trainium2 Bass kernel for nn_AU_Net (GNN message passing).

Strategy (8 NeuronCores, SPMD):
- Nodes sharded 6250/core. Weights replicated.
- Host preprocessing (graph structure only): degree/norm factors, edge
  partitioning by destination core, destination-block bucketing, int16
  gather-index packing; a separate packing per conv (conv2's table rows
  are permuted into AllGather order).
- Phase 1 builds conv1's message table replicated on every core from
  fp8 x/gx (relu(z)@Wg1 + gx@Wg1 distributed so the relu runs on the
  Activation engine and DVE only evacuates PSUM); table1 is bf16
  [N,128] rows, gathered as single 256B rows with edges stream-split
  at N/2 so indices fit int16.
- Conv2's table is fp8 e4m3, built own-shard (z1@Wg2, scaled by
  TQS*dinv) and AllGathered (6.4MB, Shared output tensors — NRT hangs
  on non-Shared collective outputs); dma_gather fetches 256B two-row
  granules and edges are stream-split by PERMUTED-ROW PARITY with the
  scatter matmul picking the correct half via the lhsT slice. The AG
  can be chunked (K_AGB) into one Shared tensor per chunk, issued as
  h2' rows complete to overlap conv1; default is a single chunk.
- Gather calls are 1024 descriptors (the 16KB SWDGE ring is a hard HW
  limit — larger rings wedge the device); desc-gen cycled over 4 SWDGE
  queues.
- GCN aggregation: scatter-add via one-hot matmul into PSUM
  (lhsT=messages, rhs=onehot), feature-major block outputs.
- dinv[src] folded into the tables; dinv[dst] applied at PSUM
  evacuation (DVE mul) with bias+relu on the Activation engine (conv2's
  evac also folds 1/TQS via the activation scale).
- The z/z0 recompute runs after the collective issues so it hides under
  it; the final dense layers are interleaved into conv2's tail.

kernel(**inputs) takes full unsharded inputs, returns the full output.
"""
import numpy as np
import ml_dtypes

import concourse.bass as bass
import concourse.bacc as bacc
import concourse.tile as tile
import concourse.mybir as mybir
from concourse import bass_utils

BF16 = mybir.dt.bfloat16
F32 = mybir.dt.float32
F8 = mybir.dt.float8e4
I16 = mybir.dt.int16

import os

NCORES = 8
M = 64             # nodes per destination block
GROUP_BLOCKS = int(os.environ.get("K_GB", "3"))
# NOTE: SWDGE rings larger than 16384B hang on real HW (49152 confirmed
# to wedge the device); keep the Bass default ring size.
MAX_RING = int(os.environ.get("K_RING", "16384"))
STRIP = int(os.environ.get("K_STRIP", "2048"))
PAD_DSTL = 99.0    # sentinel within-block id for padding edges (>= M)
# AllGather chunk boundaries, in units of completed h2' iterations (each
# h2_iter covers 4*128=512 rows of ag_in). Chunks would issue as soon as
# their ag_in rows are written, overlapping the collective with conv1's
# gathers — but NRT hangs on AllGather outputs that are not in the Shared
# address space, and the tile scheduler requires a single writer for
# Shared tensors, so the default is one chunk (single collective).
AG_BOUNDS = [int(v) for v in os.environ.get("K_AGB", "13").split(",")]
# fp8 e4m3 gather tables: halves gather DMA bytes, AllGather bytes, and
# table writes. Table values are pre-scaled by TQS (folded into the host
# dinv arrays) to keep the small-value tail out of the fp8 subnormal range;
# the inverse is folded into dinv_bc applied at PSUM evacuation.
FP8_TABLES = os.environ.get("K_FP8", "1") == "1"
TQS = 16.0
# fp8 x/gx for the phase-1 replicated table1 build only (halves its 25.6MB
# of loads); the phase-2 dense path keeps bf16 inputs.
FP8_PH1 = os.environ.get("K_FP8P1", "1") == "1"


class Meta:
    pass


# ----------------------------------------------------------------------------
# Host preprocessing (graph structure only)
# ----------------------------------------------------------------------------

def _pack_conv(stream, gidx, core, blk, within, NBLK, S=2):
    """Bucket edges by (dst core, dst block, stream) and pack the
    gather-index / within-block tables. stream in {0..S-1} selects which
    table tensor / half / parity the edge's source row lives in; gidx is
    the int16-range gather index."""
    key = (core * NBLK + blk) * S + stream
    order = np.argsort(key, kind="stable")
    gidx_s = gidx[order]
    within_s = within[order]
    key_s = key[order]

    counts = np.bincount(key_s, minlength=NCORES * NBLK * S) \
        .reshape(NCORES, NBLK, S)
    seg_start = np.concatenate([[0], np.cumsum(counts.reshape(-1))])[:-1] \
        .reshape(NCORES, NBLK, S)

    # uniform col-block counts across cores
    cb = -(-counts.max(axis=0) // 128)          # [NBLK, S]
    cb[:, 0] = np.maximum(cb[:, 0], 1)          # >=1 col-block per dst block

    ngroups = (NBLK + GROUP_BLOCKS - 1) // GROUP_BLOCKS
    groups = []
    cb_cursor = 0
    for g in range(ngroups):
        blocks = list(range(g * GROUP_BLOCKS, min((g + 1) * GROUP_BLOCKS, NBLK)))
        ginfo = {"blocks": blocks, "calls": []}
        for s in range(S):
            cbs = [int(cb[b, s]) for b in blocks]
            ginfo["calls"].append({
                "stream": s,
                "cb_total": sum(cbs),
                "cb_per_block": cbs,
                "cb_offset": cb_cursor,
            })
            cb_cursor += sum(cbs)
        groups.append(ginfo)
    CBTOT = cb_cursor

    idx16_all, dstl_all = [], []
    for c in range(NCORES):
        idx_cols = np.zeros((16, CBTOT * 8), np.int16)
        dstl = np.full((128, CBTOT), PAD_DSTL, ml_dtypes.bfloat16)
        cbi = 0
        for g in groups:
            for call in g["calls"]:
                s = call["stream"]
                n_edges_call = call["cb_total"] * 128
                e_idx = np.zeros(n_edges_call, np.int64)
                e_dstl = np.full(n_edges_call, PAD_DSTL, np.float32)
                off = 0
                for b, ncb in zip(g["blocks"], call["cb_per_block"]):
                    s0 = seg_start[c, b, s]
                    cnt = counts[c, b, s]
                    e_idx[off:off + cnt] = gidx_s[s0:s0 + cnt]
                    e_dstl[off:off + cnt] = within_s[s0:s0 + cnt]
                    off += ncb * 128
                w = e_idx.astype(np.int16).reshape(-1, 16).T
                idx_cols[:, cbi * 8:cbi * 8 + call["cb_total"] * 8] = w
                dstl[:, cbi:cbi + call["cb_total"]] = \
                    e_dstl.reshape(-1, 128).T.astype(ml_dtypes.bfloat16)
                cbi += call["cb_total"]
        idx16_all.append(np.tile(idx_cols, (8, 1)))
        dstl_all.append(dstl)

    pack = Meta()
    pack.CBTOT = CBTOT
    pack.groups = groups
    pack.max_cb = max(c["cb_total"] for g in groups for c in g["calls"])
    pack.SUBCB = min(pack.max_cb, MAX_RING // (128 * 16))
    return pack, idx16_all, dstl_all


def ag_chunk_rows(npc):
    """AG chunk row boundaries within the own shard, from AG_BOUNDS
    (units of h2 iterations = 512 rows)."""
    rows = [0]
    for b in AG_BOUNDS:
        r = min(b * 512, npc)
        if r > rows[-1]:
            rows.append(r)
    if rows[-1] != npc:
        rows.append(npc)
    return rows


def preprocess(edge_index: np.ndarray, n_nodes: int):
    N = n_nodes
    NPC = N // NCORES
    assert NPC * NCORES == N
    TSPLIT = N // 2
    assert TSPLIT < 32768 and N - TSPLIT < 32768
    NBLK = (NPC + M - 1) // M

    src = np.asarray(edge_index[0], dtype=np.int64)
    dst = np.asarray(edge_index[1], dtype=np.int64)

    deg = np.bincount(dst, minlength=N).astype(np.float64) + 1.0
    dinv = (1.0 / np.sqrt(deg)).astype(np.float32)

    loops = np.arange(N, dtype=np.int64)
    src = np.concatenate([src, loops])
    dst = np.concatenate([dst, loops])

    core = dst // NPC
    dl = dst % NPC
    blk = dl // M
    within = (dl % M).astype(np.float32)

    # table2's row order is chunk-major ([chunk][core][row]) so each AG
    # chunk writes a contiguous range; perm maps node id -> table2 row.
    rows = ag_chunk_rows(NPC)
    perm = np.empty(N, np.int64)
    off = 0
    for r0, r1 in zip(rows[:-1], rows[1:]):
        w = r1 - r0
        for k in range(NCORES):
            perm[k * NPC + r0:k * NPC + r1] = \
                off + k * w + np.arange(w, dtype=np.int64)
        off += NCORES * w
    assert off == N

    # conv1 gathers 256B bf16 rows from table1 split in halves at TSPLIT
    st1 = (src >= TSPLIT).astype(np.int64)
    pack1, idx16_1, dstl_1 = _pack_conv(st1, src - st1 * TSPLIT,
                                        core, blk, within, NBLK)
    # conv2 gathers from one Shared table tensor per AG chunk; stream id =
    # (chunk, parity) for fp8 (256B two-row granules) or chunk for bf16.
    p = perm[src]
    nch = len(rows) - 1
    bounds = np.array([NCORES * r for r in rows], np.int64)
    ci = np.searchsorted(bounds, p, side="right") - 1
    p_rel = p - bounds[ci]
    if FP8_TABLES:
        st2 = 2 * ci + (p_rel & 1)
        g2 = p_rel >> 1
        S2 = 2 * nch
    else:
        st2 = ci
        g2 = p_rel
        S2 = nch
    assert int(g2.max()) < 32768
    pack2, idx16_2, dstl_2 = _pack_conv(st2, g2, core, blk, within,
                                        NBLK, S=S2)

    meta = Meta()
    meta.N, meta.NPC, meta.NBLK = N, NPC, NBLK
    meta.TSPLIT = TSPLIT
    meta.pack1, meta.pack2 = pack1, pack2
    meta.CBTOT = pack1.CBTOT + pack2.CBTOT
    meta.ag_rows = rows
    meta.n_chunks = nch
    meta.SCRATCH = max(16384, max(pack1.SUBCB, pack2.SUBCB) * 128 * 16)

    # fp8 table2 values are pre-scaled by TQS (folded into dinv_own) to
    # keep the small-value tail out of fp8 subnormals; the inverse 1/TQS
    # is a constant activation scale at conv2's PSUM evacuation.
    tqs = TQS if FP8_TABLES else 1.0
    dinv_own_all, dinv_bc_all = [], []
    for c in range(NCORES):
        c0n = c * NPC
        nown = (NPC + 127) // 128
        dvals = dinv[c0n:c0n + NPC]
        d_own_flat = np.zeros(nown * 128, np.float32)
        d_own_flat[:NPC] = dvals * tqs
        dinv_own_all.append(np.ascontiguousarray(
            d_own_flat.reshape(nown, 128).T))
        dinv_bc_all.append(np.broadcast_to(
            dvals.astype(ml_dtypes.bfloat16)[None, :],
            (128, NPC)).copy())

    nnm = (N + 127) // 128
    d_nm_flat = np.zeros(nnm * 128, np.float32)
    d_nm_flat[:N] = dinv
    dinv_nm = np.ascontiguousarray(d_nm_flat.reshape(nnm, 128).T)

    arrays = {
        "idx16a": idx16_1,
        "idx16b": idx16_2,
        "dstla": dstl_1,
        "dstlb": dstl_2,
        "dinv_own_nm": dinv_own_all,
        "dinv_bc": dinv_bc_all,
        "dinv_nm": dinv_nm,
    }
    return meta, arrays


# ----------------------------------------------------------------------------
# Device program
# ----------------------------------------------------------------------------

def _declare_io(nc, meta):
    N, NPC = meta.N, meta.NPC
    NNM = (N + 127) // 128
    NOWN = (NPC + 127) // 128

    def din(name, shape, dt):
        return nc.dram_tensor(name, shape, dt, kind="ExternalInput").ap()

    io = {}
    ph1dt = F8 if FP8_PH1 else BF16
    io["xT"] = din("xT", [128, N], ph1dt)
    io["gxT"] = din("gxT", [128, N], ph1dt)
    io["xTo"] = din("xTo", [128, NPC], BF16)
    io["gxTo"] = din("gxTo", [128, NPC], BF16)
    io["idx16a"] = din("idx16a", [128, meta.pack1.CBTOT * 8], I16)
    io["idx16b"] = din("idx16b", [128, meta.pack2.CBTOT * 8], I16)
    io["dstla"] = din("dstla", [128, meta.pack1.CBTOT], BF16)
    io["dstlb"] = din("dstlb", [128, meta.pack2.CBTOT], BF16)
    io["dinv_nm"] = din("dinv_nm", [128, NNM], F32)
    io["dinv_own_nm"] = din("dinv_own_nm", [128, NOWN], F32)
    io["dinv_bc"] = din("dinv_bc", [128, NPC], BF16)

    wspec = [("W1a", 128), ("W1b", 128), ("Wdr", 128), ("Wg1", 128),
             ("Wg2", 128), ("W2a", 128), ("W2b", 128), ("W2c", 128),
             ("W3", 64)]
    io["wspec"] = wspec
    io["wins"] = {nm: din(nm, [128, fo], BF16) for nm, fo in wspec}
    io["b1row"] = din("b1row", [1, 128], ph1dt)
    if FP8_PH1:
        io["W1a8"] = din("W1a8", [128, 128], F8)
        io["W1b8"] = din("W1b8", [128, 128], F8)
        io["Wg1_8"] = din("Wg1_8", [128, 128], F8)
    io["Wo"] = din("Wo", [64, 10], BF16)
    bspec = [("b1", 128), ("bdr", 128), ("bg1", 128), ("bg2", 128),
             ("b2", 128), ("b3", 64), ("bo", 10)]
    io["bspec"] = bspec
    io["bins"] = {nm: din(nm, [d, 1], F32) for nm, d in bspec}
    return io


def build_program(meta):
    N, NPC, NBLK = meta.N, meta.NPC, meta.NBLK
    TSPLIT = meta.TSPLIT
    NNM = (N + 127) // 128
    NOWN = (NPC + 127) // 128
    LOADW = STRIP       # phase-1 load width (per-DMA columns)
    # deeper phase-1 load buffering when conv2 has only 2 streams
    PH1B = 3 if meta.n_chunks == 1 else 2
    TDT = F8 if FP8_TABLES else BF16

    nc = bacc.Bacc("TRN2", target_bir_lowering=False, debug=False,
                   num_devices=NCORES, num_swdge_queues=4,
                   dynamic_dma_scratch_size=meta.SCRATCH)
    io = _declare_io(nc, meta)
    outT = nc.dram_tensor("outT", [10, NPC], F32, kind="ExternalOutput").ap()

    ADD, MAX, MULT = (mybir.AluOpType.add, mybir.AluOpType.max,
                      mybir.AluOpType.mult)
    RELU = mybir.ActivationFunctionType.Relu
    COPY = mybir.ActivationFunctionType.Copy

    with tile.TileContext(nc) as tc:
        with tc.tile_pool(name="res", bufs=1) as res, \
             tc.tile_pool(name="dram", bufs=1, space="DRAM") as dram, \
             tc.tile_pool(name="work", bufs=1) as work, \
             tc.tile_pool(name="pbig", bufs=2, space="PSUM") as pbig, \
             tc.tile_pool(name="pconv", bufs=3, space="PSUM") as pconv:

            # ---------------- residents ----------------
            zT = res.tile([128, NPC], BF16)
            z0T = res.tile([128, NPC], BF16)
            z1T = res.tile([128, NPC], BF16)
            z2T = res.tile([128, NPC], BF16)
            dstla_s = res.tile([128, meta.pack1.CBTOT], BF16)
            dstlb_s = res.tile([128, meta.pack2.CBTOT], BF16)
            dinv_nm_s = res.tile([128, NNM], F32)
            dinv_own_s = res.tile([128, NOWN], F32)
            dinv_bc_s = res.tile([128, NPC], BF16)
            iota64 = res.tile([128, M], BF16)

            nc.sync.dma_start(out=dstla_s[:], in_=io["dstla"][:])
            nc.sync.dma_start(out=dstlb_s[:], in_=io["dstlb"][:])
            nc.sync.dma_start(out=dinv_nm_s[:], in_=io["dinv_nm"][:])
            nc.sync.dma_start(out=dinv_own_s[:], in_=io["dinv_own_nm"][:])
            nc.sync.dma_start(out=dinv_bc_s[:], in_=io["dinv_bc"][:])
            nc.gpsimd.iota(iota64[:], pattern=[[1, M]], base=0,
                           channel_multiplier=0,
                           allow_small_or_imprecise_dtypes=True)

            wt = {}
            for nm, fo in io["wspec"]:
                t = res.tile([128, fo], BF16, name=f"w_{nm}")
                nc.sync.dma_start(out=t[:], in_=io["wins"][nm][:])
                wt[nm] = t
            wo_t = res.tile([64, 10], BF16)
            nc.sync.dma_start(out=wo_t[:], in_=io["Wo"][:])
            bias = {}
            for nm, d in io["bspec"]:
                t = res.tile([d, 1], F32, name=f"b_{nm}")
                nc.sync.dma_start(out=t[:], in_=io["bins"][nm][:])
                bias[nm] = t

            # ---------------- DRAM internals ----------------
            table1 = dram.tile([N, 128], BF16)
            ag_in = dram.tile([NPC, 128], TDT)
            # one Shared tensor per AG chunk: each is written by exactly
            # one AllGather (the tile scheduler requires a single writer
            # for Shared tensors, and NRT requires Shared outputs)
            ag_rows = meta.ag_rows
            t2c = []
            for ci in range(meta.n_chunks):
                w = NCORES * (ag_rows[ci + 1] - ag_rows[ci])
                shp = [w // 2, 256] if FP8_TABLES else [w, 128]
                t2c.append(dram.tile(shp, TDT, addr_space="Shared",
                                     name=f"table2_{ci}"))

            ph1dt = F8 if FP8_PH1 else BF16
            b1row = res.tile([1, 128], ph1dt)
            nc.sync.dma_start(out=b1row[:], in_=io["b1row"][:])
            ones_row = res.tile([1, 512], ph1dt)
            nc.vector.memset(ones_row[:], 1.0)
            if FP8_PH1:
                w1a8 = res.tile([128, 128], F8)
                w1b8 = res.tile([128, 128], F8)
                wg1_8 = res.tile([128, 128], F8)
                nc.sync.dma_start(out=w1a8[:], in_=io["W1a8"][:])
                nc.sync.dma_start(out=w1b8[:], in_=io["W1b8"][:])
                nc.sync.dma_start(out=wg1_8[:], in_=io["Wg1_8"][:])
            else:
                w1a8, w1b8, wg1_8 = wt["W1a"], wt["W1b"], wt["Wg1"]

            # ---------------- phase 1: replicated table1 build -------------
            CW = 512
            nload = (N + LOADW - 1) // LOADW
            nhv = LOADW // 128
            for si in range(nload):
                l0 = si * LOADW
                lcols = min(LOADW, N - l0)
                xs = work.tile([128, LOADW], ph1dt, tag="xs", bufs=PH1B)
                gs = work.tile([128, LOADW], ph1dt, tag="gs", bufs=PH1B)
                nc.sync.dma_start(out=xs[:, :lcols],
                                  in_=io["xT"][:, l0:l0 + lcols])
                nc.sync.dma_start(out=gs[:, :lcols],
                                  in_=io["gxT"][:, l0:l0 + lcols])
                hv = work.tile([128, nhv, 128], BF16, tag="hv", bufs=3)
                for h0 in range(0, lcols, CW):
                    cols = min(CW, lcols - h0)
                    s0 = l0 + h0
                    pz = pbig.tile([128, CW], F32, tag="big")
                    nc.tensor.matmul(out=pz[:, :cols], lhsT=w1a8[:],
                                     rhs=xs[:, h0:h0 + cols],
                                     start=True, stop=False)
                    nc.tensor.matmul(out=pz[:, :cols], lhsT=w1b8[:],
                                     rhs=gs[:, h0:h0 + cols],
                                     start=False, stop=False)
                    nc.tensor.matmul(out=pz[:, :cols], lhsT=b1row[:],
                                     rhs=ones_row[:, :cols],
                                     start=False, stop=True)
                    # distribute (relu(z)+gx)@Wg1 = relu(z)@Wg1 + gx@Wg1:
                    # relu runs on Act, the gx term is an extra PE matmul,
                    # and DVE only does the PSUM evacuations.
                    a_s = work.tile([128, CW], BF16, tag="as", bufs=3)
                    nc.scalar.activation(out=a_s[:, :cols],
                                         in_=pz[:, :cols], func=RELU)
                    pn = pconv.tile([128, CW], F32, tag="pnm", bufs=3)
                    nchunk = (cols + 127) // 128
                    for k in range(nchunk):
                        mcols = min(128, cols - k * 128)
                        nc.tensor.matmul(
                            out=pn[:mcols, k * 128:k * 128 + 128],
                            lhsT=a_s[:, k * 128:k * 128 + mcols],
                            rhs=wt["Wg1"][:], start=True, stop=False)
                        gc0 = h0 + k * 128
                        nc.tensor.matmul(
                            out=pn[:mcols, k * 128:k * 128 + 128],
                            lhsT=gs[:, gc0:gc0 + mcols],
                            rhs=wg1_8[:], start=False, stop=True)
                        gchunk = (s0 + k * 128) // 128
                        kk = h0 // 128 + k
                        nc.vector.tensor_scalar(
                            out=hv[:mcols, kk, :],
                            in0=pn[:mcols, k * 128:k * 128 + 128],
                            scalar1=dinv_nm_s[:mcols, gchunk:gchunk + 1],
                            scalar2=None, op0=MULT)
                c128 = (lcols // 128) * 128
                eng = nc.scalar if si % 2 == 0 else nc.sync
                if c128:
                    eng.dma_start(
                        out=table1[l0:l0 + c128, :].rearrange(
                            "(c p) f -> p c f", p=128),
                        in_=hv[:, :c128 // 128, :])
                if lcols - c128:
                    eng.dma_start(
                        out=table1[l0 + c128:l0 + lcols, :],
                        in_=hv[:lcols - c128, c128 // 128, :])

            # ---------------- conv phases ----------------
            def conv_phase(pack, idx_io, dstl_s, src_aps, msg_dt, msg_w,
                           lh_off, out_res, bias_col, tagp, evac_scale=1.0,
                           tail_fn=None):
                """src_aps[s]: gather source AP for stream s; msg_w: gathered
                elements per edge (128 bf16 row / 256 fp8 two-row granule);
                lh_off[s]: element offset of the 128-wide message slice in
                the gathered granule for stream s."""
                groups = pack.groups
                max_cb = pack.max_cb
                SUBCB = pack.SUBCB
                nstreams = len(src_aps)
                for gi, g in enumerate(groups):
                    msgs, ohs = [], []
                    for call in g["calls"]:
                        s = call["stream"]
                        ncb = call["cb_total"]
                        coff = call["cb_offset"]
                        it = work.tile([128, max_cb * 8], I16,
                                       tag=f"{tagp}idx{s}", bufs=2)
                        nc.sync.dma_start(
                            out=it[:, :ncb * 8],
                            in_=idx_io[:, coff * 8:(coff + ncb) * 8])
                        mt = work.tile([128, max_cb, msg_w], msg_dt,
                                       tag=f"{tagp}msg{s}", bufs=2)
                        for k0 in range(0, ncb, SUBCB):
                            kcb = min(SUBCB, ncb - k0)
                            nc.gpsimd.dma_gather(
                                out_ap=mt[:, k0:k0 + kcb, :],
                                in_ap=src_aps[s],
                                idxs_ap=it[:, k0 * 8:(k0 + kcb) * 8],
                                num_idxs=kcb * 128,
                                num_idxs_reg=kcb * 128,
                                elem_size=msg_w,
                            )
                        oh = work.tile([128, max_cb, M], msg_dt,
                                       tag=f"{tagp}oh{s}", bufs=2)
                        iota_b = iota64[:].unsqueeze(1) \
                            .broadcast_to([128, ncb, M])
                        dstl_b = dstl_s[:, coff:coff + ncb].unsqueeze(2) \
                            .broadcast_to([128, ncb, M])
                        nc.vector.tensor_tensor(out=oh[:, :ncb, :],
                                                in0=iota_b, in1=dstl_b,
                                                op=mybir.AluOpType.is_equal)
                        msgs.append(mt)
                        ohs.append(oh)

                    colpos = [0] * nstreams
                    for bi, b in enumerate(g["blocks"]):
                        pc = pconv.tile([128, M], F32, tag="pcv", bufs=3)
                        tot = sum(call["cb_per_block"][bi]
                                  for call in g["calls"])
                        done = 0
                        for s, call in enumerate(g["calls"]):
                            ncb_s = call["cb_per_block"][bi]
                            for k in range(ncb_s):
                                col = colpos[s] + k
                                nc.tensor.matmul(
                                    out=pc[:],
                                    lhsT=msgs[s][:, col,
                                                 lh_off[s]:lh_off[s] + 128],
                                    rhs=ohs[s][:, col, :],
                                    start=(done == 0),
                                    stop=(done == tot - 1))
                                done += 1
                            colpos[s] += ncb_s
                        mb = min(M, NPC - b * M)
                        bcol = b * M
                        tmpv = work.tile([128, M], BF16, tag="cevac", bufs=4)
                        nc.vector.tensor_mul(
                            out=tmpv[:, :mb], in0=pc[:, :mb],
                            in1=dinv_bc_s[:, bcol:bcol + mb])
                        nc.scalar.activation(
                            out=out_res[:, bcol:bcol + mb],
                            in_=tmpv[:, :mb],
                            func=RELU, scale=evac_scale, bias=bias_col[:, :1])
                    if tail_fn is not None:
                        tail_fn(gi)

            # ---- h2' own-shard transform, interleaved into conv1 tail ----
            def h2_iter(k0):
                kn = min(4, NOWN - k0)
                hv2 = work.tile([128, 4, 128], TDT, tag="hv2", bufs=3)
                for kk in range(kn):
                    k = k0 + kk
                    mcols = min(128, NPC - k * 128)
                    pn2 = pconv.tile([128, CW], F32, tag="pnm", bufs=3)
                    nc.tensor.matmul(out=pn2[:mcols, :128],
                                     lhsT=z1T[:, k * 128:k * 128 + mcols],
                                     rhs=wt["Wg2"][:], start=True, stop=True)
                    if kk % 2 == 0:
                        nc.vector.tensor_scalar(
                            out=hv2[:mcols, kk, :], in0=pn2[:mcols, :128],
                            scalar1=dinv_own_s[:mcols, k:k + 1],
                            scalar2=None, op0=MULT)
                    else:
                        nc.scalar.activation(
                            out=hv2[:mcols, kk, :], in_=pn2[:mcols, :128],
                            func=COPY, scale=dinv_own_s[:mcols, k:k + 1])
                r0 = k0 * 128
                rows = min(NPC - r0, kn * 128)
                r128 = (rows // 128) * 128
                if r128:
                    nc.sync.dma_start(
                        out=ag_in[r0:r0 + r128, :].rearrange(
                            "(c p) f -> p c f", p=128),
                        in_=hv2[:, :r128 // 128, :])
                if rows - r128:
                    nc.sync.dma_start(
                        out=ag_in[r0 + r128:r0 + rows, :],
                        in_=hv2[:rows - r128, r128 // 128, :])

            h2_emitted = [0]
            ag_issued = [0]

            def maybe_issue_ag():
                while (ag_issued[0] < len(ag_rows) - 1
                       and h2_emitted[0] * 128 >= ag_rows[ag_issued[0] + 1]):
                    r0 = ag_rows[ag_issued[0]]
                    r1 = ag_rows[ag_issued[0] + 1]
                    nc.gpsimd.collective_compute(
                        "AllGather",
                        mybir.AluOpType.bypass,
                        replica_groups=[list(range(NCORES))],
                        ins=[ag_in[r0:r1, :]],
                        outs=[t2c[ag_issued[0]][:]],
                    )
                    ag_issued[0] += 1

            def conv1_tail(gi):
                done_cols = (gi + 1) * GROUP_BLOCKS * M
                while (h2_emitted[0] * 128 < NPC
                       and (h2_emitted[0] + 4) * 128 <= done_cols):
                    h2_iter(h2_emitted[0])
                    h2_emitted[0] += 4
                maybe_issue_ag()

            conv_phase(meta.pack1, io["idx16a"], dstla_s,
                       [table1[0:TSPLIT, :], table1[TSPLIT:N, :]],
                       BF16, 128, [0, 0], z1T, bias["bg1"], "a",
                       tail_fn=conv1_tail)
            while h2_emitted[0] * 128 < NPC:
                h2_iter(h2_emitted[0])
                h2_emitted[0] += 4
            maybe_issue_ag()
            assert ag_issued[0] == len(ag_rows) - 1

            FW = 512
            # ------------- phase 2: own-slice z and z0 (hidden under AG) ----
            nown_strips = (NPC + FW - 1) // FW
            for si in range(nown_strips):
                s0 = si * FW
                cols = min(FW, NPC - s0)
                xs2 = work.tile([128, FW], BF16, tag="xs2", bufs=2)
                gs2 = work.tile([128, FW], BF16, tag="gs2", bufs=2)
                nc.sync.dma_start(out=xs2[:, :cols], in_=io["xTo"][:, s0:s0 + cols])
                nc.sync.dma_start(out=gs2[:, :cols], in_=io["gxTo"][:, s0:s0 + cols])
                pz2 = pbig.tile([128, FW], F32, tag="big")
                nc.tensor.matmul(out=pz2[:, :cols], lhsT=wt["W1a"][:],
                                 rhs=xs2[:, :cols], start=True, stop=False)
                nc.tensor.matmul(out=pz2[:, :cols], lhsT=wt["W1b"][:],
                                 rhs=gs2[:, :cols], start=False, stop=True)
                nc.scalar.activation(out=zT[:, s0:s0 + cols],
                                     in_=pz2[:, :cols],
                                     func=RELU, bias=bias["b1"][:, :1])
                pz0 = pbig.tile([128, FW], F32, tag="big")
                nc.tensor.matmul(out=pz0[:, :cols], lhsT=wt["Wdr"][:],
                                 rhs=zT[:, s0:s0 + cols], start=True, stop=True)
                nc.vector.tensor_scalar(out=z0T[:, s0:s0 + cols],
                                        in0=pz0[:, :cols],
                                        scalar1=bias["bdr"][:, :1],
                                        scalar2=None, op0=ADD)

            # ---------------- conv2 with interleaved final dense ------------
            def final_strip(si):
                s0 = si * FW
                cols = min(FW, NPC - s0)
                if cols <= 0:
                    return
                pcc = pbig.tile([128, FW], F32, tag="big")
                nc.tensor.matmul(out=pcc[:, :cols], lhsT=wt["W2a"][:],
                                 rhs=zT[:, s0:s0 + cols],
                                 start=True, stop=False)
                nc.tensor.matmul(out=pcc[:, :cols], lhsT=wt["W2b"][:],
                                 rhs=z1T[:, s0:s0 + cols],
                                 start=False, stop=False)
                nc.tensor.matmul(out=pcc[:, :cols], lhsT=wt["W2c"][:],
                                 rhs=z2T[:, s0:s0 + cols],
                                 start=False, stop=True)
                zc = work.tile([128, FW], BF16, tag="zc", bufs=2)
                nc.scalar.activation(out=zc[:, :cols], in_=pcc[:, :cols],
                                     func=RELU, bias=bias["b2"][:, :1])
                u = work.tile([128, FW], BF16, tag="u", bufs=2)
                nc.vector.tensor_add(out=u[:, :cols], in0=zc[:, :cols],
                                     in1=z0T[:, s0:s0 + cols])
                p6 = pbig.tile([64, FW], F32, tag="big")
                nc.tensor.matmul(out=p6[:, :cols], lhsT=wt["W3"][:],
                                 rhs=u[:, :cols], start=True, stop=True)
                u6 = work.tile([64, FW], BF16, tag="u6", bufs=2)
                nc.scalar.activation(out=u6[:, :cols], in_=p6[:, :cols],
                                     func=RELU, bias=bias["b3"][:, :1])
                po = pbig.tile([10, FW], F32, tag="big")
                nc.tensor.matmul(out=po[:, :cols], lhsT=wo_t[:],
                                 rhs=u6[:, :cols], start=True, stop=True)
                ofin = work.tile([10, FW], F32, tag="ofin", bufs=2)
                nc.vector.tensor_scalar(out=ofin[:, :cols], in0=po[:, :cols],
                                        scalar1=bias["bo"][:, :1],
                                        scalar2=None, op0=ADD)
                nc.sync.dma_start(out=outT[:, s0:s0 + cols],
                                  in_=ofin[:, :cols])

            nown_strips_f = (NPC + FW - 1) // FW
            emitted = [0]

            def conv2_tail(gi):
                # final strip si needs z2T cols < (si+1)*FW, i.e. conv2
                # blocks through (si+1)*FW//M; group gi finished block
                # (gi+1)*GROUP_BLOCKS-1 = cols (gi+1)*GROUP_BLOCKS*M.
                done_cols = (gi + 1) * GROUP_BLOCKS * M
                while (emitted[0] < nown_strips_f
                       and (emitted[0] + 1) * FW <= done_cols):
                    final_strip(emitted[0])
                    emitted[0] += 1

            if FP8_TABLES:
                src_aps2 = [t2c[ci][:] for ci in range(meta.n_chunks)
                            for _ in (0, 1)]
                lh_off2 = [off for _ in range(meta.n_chunks)
                           for off in (0, 128)]
                conv_phase(meta.pack2, io["idx16b"], dstlb_s,
                           src_aps2, F8, 256, lh_off2,
                           z2T, bias["bg2"], "b", evac_scale=1.0 / TQS,
                           tail_fn=conv2_tail)
            else:
                src_aps2 = [t2c[ci][:] for ci in range(meta.n_chunks)]
                conv_phase(meta.pack2, io["idx16b"], dstlb_s,
                           src_aps2, BF16, 128, [0] * meta.n_chunks,
                           z2T, bias["bg2"], "b",
                           tail_fn=conv2_tail)
            while emitted[0] < nown_strips_f:
                final_strip(emitted[0])
                emitted[0] += 1

    _assign_gather_queues(nc)
    nc.compile()
    return nc


def _assign_gather_queues(nc):
    """Post-scheduling: route each gather to SWDGE queue (lane % 4), where
    lane is the DMASW semaphore lane Tile assigned. Lanes then never share
    a queue's increments, keeping per-lane FIFO semantics sound while the
    4 queues generate descriptors in parallel."""
    for bb in nc.main_func.blocks:
        for inst in bb.instructions:
            if isinstance(inst, mybir.InstDMAGatherAnt):
                si = inst.sync_info
                if not si or not si.on_update:
                    continue
                nm = si.on_update[0].ant_name or ""
                if nm.startswith("DMASW"):
                    lane = int(nm[5:].split("_")[0])
                    inst.queue_num = lane % 4


def build_skeleton(meta):
    """Same I/O signature as build_program, trivial body (floor measure)."""
    NPC = meta.NPC
    nc = bacc.Bacc("TRN2", target_bir_lowering=False, debug=False,
                   num_devices=NCORES, num_swdge_queues=4,
                   dynamic_dma_scratch_size=meta.SCRATCH)
    _declare_io(nc, meta)
    outT = nc.dram_tensor("outT", [10, NPC], F32, kind="ExternalOutput").ap()
    with tile.TileContext(nc) as tc:
        with tc.tile_pool(name="w", bufs=1) as w:
            t = w.tile([10, NPC], F32)
            nc.vector.memset(t[:], 0.0)
            nc.sync.dma_start(out=outT[:], in_=t[:])
    nc.compile()
    return nc


# ----------------------------------------------------------------------------
# Input packing + entry point
# ----------------------------------------------------------------------------

def pack_inputs(inputs, meta, arrays):
    NPC = meta.NPC
    bf = ml_dtypes.bfloat16
    f8 = ml_dtypes.float8_e4m3fn

    x = np.asarray(inputs["x"], np.float32)
    gx = np.asarray(inputs["gx"], np.float32)
    xT = np.ascontiguousarray(x.T.astype(bf))
    gxT = np.ascontiguousarray(gx.T.astype(bf))
    ph1 = f8 if FP8_PH1 else bf

    W1 = np.asarray(inputs["W1"], np.float32)
    W2 = np.asarray(inputs["W2"], np.float32)
    weights = {
        "W1a": W1[:128].astype(bf), "W1b": W1[128:].astype(bf),
        "Wdr": np.asarray(inputs["Wdr"], np.float32).astype(bf),
        "Wg1": np.asarray(inputs["Wg1"], np.float32).astype(bf),
        "Wg2": np.asarray(inputs["Wg2"], np.float32).astype(bf),
        "W2a": W2[:128].astype(bf), "W2b": W2[128:256].astype(bf),
        "W2c": W2[256:].astype(bf),
        "W3": np.asarray(inputs["W3"], np.float32).astype(bf),
        "Wo": np.asarray(inputs["Wo"], np.float32).astype(bf),
    }
    biases = ["b1", "bdr", "bg1", "bg2", "b2", "b3", "bo"]

    xTp = np.ascontiguousarray(x.T.astype(ph1))
    gxTp = np.ascontiguousarray(gx.T.astype(ph1))

    in_maps = []
    for c in range(NCORES):
        m = {
            "xT": xTp, "gxT": gxTp,
            "xTo": np.ascontiguousarray(xT[:, c * NPC:(c + 1) * NPC]),
            "gxTo": np.ascontiguousarray(gxT[:, c * NPC:(c + 1) * NPC]),
            "idx16a": arrays["idx16a"][c],
            "idx16b": arrays["idx16b"][c],
            "dstla": arrays["dstla"][c],
            "dstlb": arrays["dstlb"][c],
            "dinv_nm": arrays["dinv_nm"],
            "dinv_own_nm": arrays["dinv_own_nm"][c],
            "dinv_bc": arrays["dinv_bc"][c],
        }
        for k, v in weights.items():
            m[k] = np.ascontiguousarray(v)
        for k in biases:
            m[k] = np.ascontiguousarray(
                np.asarray(inputs[k], np.float32).reshape(-1, 1))
        m["b1row"] = np.ascontiguousarray(
            np.asarray(inputs["b1"], np.float32).reshape(1, 128).astype(ph1))
        if FP8_PH1:
            m["W1a8"] = np.ascontiguousarray(W1[:128].astype(f8))
            m["W1b8"] = np.ascontiguousarray(W1[128:].astype(f8))
            m["Wg1_8"] = np.ascontiguousarray(
                np.asarray(inputs["Wg1"], np.float32).astype(f8))
        in_maps.append(m)
    return in_maps


_CACHE = {}


def prepare(inputs):
    edge_index = np.asarray(inputs["edge_index"])
    n = int(np.asarray(inputs["x"]).shape[0])
    key = (n, edge_index.shape[1], hash(edge_index.tobytes()))
    if key not in _CACHE:
        meta, arrays = preprocess(edge_index, n)
        nc = build_program(meta)
        _CACHE.clear()
        _CACHE[key] = (nc, meta, arrays)
    return _CACHE[key]


def kernel(**inputs) -> np.ndarray:
    nc, meta, arrays = prepare(inputs)
    in_maps = pack_inputs(inputs, meta, arrays)
    res = bass_utils.run_bass_kernel_spmd(
        nc, in_maps, core_ids=list(range(NCORES)))
    out = np.concatenate(
        [res.results[c]["outT"].T for c in range(NCORES)], axis=0)
    return out.astype(np.float32)

